# revision 55
# baseline (speedup 1.0000x reference)
"""Trainium2 Bass kernel for nn_BranchingQNetwork (12-branch dueling Q-MLP).

Strategy: data-parallel over batch (8 cores x 1024 rows). bf16 weights and
activations (tolerance 2e-2; bf16 lands ~5e-3). All weights for a branch are
SBUF-resident and double-buffered, streamed from HBM once per core, so the PE
never waits on weight DMAs. L2/L3 run m-outer/k-inner accumulation chains so
PSUM drains overlap the next chain's matmuls. The next iteration's L1
matmuls are interleaved into the current L2 window (h1 double-buffered). The
per-branch input px is host-packed as [node | group_br | ones | zeros] with
K padded to 128 so every matmul has a uniform [128, 128] stationary operand
(keeps the PE weight-buffer pull-ahead clean) and the L1 bias rides in the
ones row. The dueling head is folded host-side into a single [512, 12]
matrix and computed action-major (stationary = Wq k-tiles, moving = h3); the
[12, LB] output is transposed back to batch-major on the host.
"""
import sys

sys.path.insert(0, "/opt/trn_rl_repo")

import numpy as np
import ml_dtypes

# problem dims (hardcoded per harness contract)
B = 8192
OBS = 249
NB = 12
NA = 11
NODE = 45
GRP = 17
D0 = 62
D1 = 2048
D2 = 1024
D3 = 512

NCORES = 8
LB = B // NCORES     # local batch per core
BT = 512             # batch tile (one PSUM bank of fp32)
NBT = LB // BT
M1 = D1 // 128       # 16 L1 output tiles
M2 = D2 // 128       # 8
K2 = D1 // 128       # 16
M3 = D3 // 128       # 4
K3 = D2 // 128       # 8
KH = D3 // 128       # 4
NAP = 12             # head width padded even

_NC_CACHE = {}
LAST_RESULT = None


def _build_nc():
    if "nc" in _NC_CACHE:
        return _NC_CACHE["nc"]
    from concourse import bacc
    import concourse.mybir as mybir
    import concourse.tile as tile

    f32 = mybir.dt.float32
    bf16 = mybir.dt.bfloat16
    Relu = mybir.ActivationFunctionType.Relu
    ADD = mybir.AluOpType.add
    MAX = mybir.AluOpType.max

    nc = bacc.Bacc("TRN2")

    xE_d = nc.declare_dram_parameter("xE", [NB, 128, LB], bf16, isOutput=False)
    W1_d = nc.declare_dram_parameter("W1p", [NB, 128, D1], bf16, isOutput=False)
    W2_d = nc.declare_dram_parameter("W2p", [NB, 128, M2, K2, 128], bf16, isOutput=False)
    W3_d = nc.declare_dram_parameter("W3p", [NB, 128, M3, K3, 128], bf16, isOutput=False)
    Wq_d = nc.declare_dram_parameter("Wqp", [NB, 128, KH, NAP], bf16, isOutput=False)
    b_d = nc.declare_dram_parameter("bp", [NB, 128, M2 + M3], f32, isOutput=False)
    bq_d = nc.declare_dram_parameter("bqp", [NB, NAP, 1], f32, isOutput=False)
    out_d = nc.declare_dram_parameter("out", [NB, NAP, LB], f32, isOutput=True)

    with tile.TileContext(nc) as tc:
        with (
            tc.tile_pool(name="w1p", bufs=2) as w1p,
            tc.tile_pool(name="w2p", bufs=2) as w2p,
            tc.tile_pool(name="w3p", bufs=2) as w3p,
            tc.tile_pool(name="wqp", bufs=2) as wqp,
            tc.tile_pool(name="bbp", bufs=2) as bbp,
            tc.tile_pool(name="pxp", bufs=4) as pxp,
            tc.tile_pool(name="h1p", bufs=2) as h1p,
            tc.tile_pool(name="h3p", bufs=2) as h3p,
            tc.tile_pool(name="actp", bufs=1) as actp,
            tc.tile_pool(name="qsp", bufs=2) as qsp,
            tc.tile_pool(name="ps1", bufs=3, space="PSUM") as ps1,
            tc.tile_pool(name="ps2", bufs=5, space="PSUM") as ps2,
        ):
            h2 = actp.tile([128, K3, BT], bf16, tag="h2")

            loaded = {}
            pxs = {}
            h1s = {}
            h3s = {}
            qss = {}
            iters = [(br, bt) for br in range(NB) for bt in range(NBT)]

            def load_branch(br):
                w1t = w1p.tile([128, D1], bf16, tag="w1", name=f"w1_{br}")
                if br == 0:
                    # rows 64-127 of the padded W1 are zeros: memset them
                    # on-chip and DMA only rows 0-63, halving the startup
                    # critical-path bytes. Column-chunked; chunk 0 rides the
                    # (fast cold-start) gpsimd SWDGE path so the first L1
                    # matmul gates on 64KB of DMA.
                    nc.vector.memset(w1t[64:128, :], 0.0)
                    for c in range(4):
                        eng = nc.gpsimd if c == 0 else nc.sync
                        eng.dma_start(
                            w1t[0:64, c * 512:(c + 1) * 512],
                            W1_d[br][0:64, c * 512:(c + 1) * 512],
                        )
                else:
                    nc.scalar.dma_start(w1t[:], W1_d[br])
                btile = bbp.tile([128, M2 + M3], f32, tag="b", name=f"b_{br}")
                nc.scalar.dma_start(btile[:], b_d[br])
                bqt = bbp.tile([NAP, 1], f32, tag="bq", name=f"bq_{br}")
                nc.scalar.dma_start(bqt[:], bq_d[br])
                wqt = wqp.tile([128, KH, NAP], bf16, tag="wq", name=f"wq_{br}")
                nc.scalar.dma_start(wqt[:], Wq_d[br])
                w2t = w2p.tile([128, M2, K2, 128], bf16, tag="w2", name=f"w2_{br}")
                nm = M2 if br == 0 else M2 // 2
                if br == 0:
                    # cold-start gpsimd ring in order of first use: px1
                    # (first L1 injection), W2 m0 k-halved (first L2 chain
                    # starts on half-arrival), px2, then m1
                    load_px(1, eng=nc.gpsimd, half=True)
                    nc.gpsimd.dma_start(w2t[:, 0, 0:K2 // 2],
                                        W2_d[br, :, 0, 0:K2 // 2])
                    nc.gpsimd.dma_start(w2t[:, 0, K2 // 2:K2],
                                        W2_d[br, :, 0, K2 // 2:K2])
                    load_px(2, eng=nc.gpsimd, half=True)
                    nc.gpsimd.dma_start(w2t[:, 1], W2_d[br, :, 1])
                    for m in range(2, M2):
                        nc.sync.dma_start(w2t[:, m], W2_d[br, :, m])
                else:
                    for m in range(M2 // 2):
                        nc.sync.dma_start(w2t[:, m], W2_d[br, :, m])
                w3t = w3p.tile([128, M3, K3, 128], bf16, tag="w3", name=f"w3_{br}")
                # branch 0's W3 rides the sync queue behind W2 so it doesn't
                # steal HBM bandwidth from the critical first L1/L2 weights
                (nc.sync if br == 0 else nc.gpsimd).dma_start(w3t[:], W3_d[br])
                loaded[br] = (w1t, w2t, w3t, wqt, btile, bqt)

            def load_branch_rest(br):
                # second half of the W2 prefetch, deferred one iteration to
                # smooth the HBM burst (the stack is shared with a neighbor
                # core running the same phase-drifting schedule)
                w2t = loaded[br][1]
                for m in range(M2 // 2, M2):
                    nc.sync.dma_start(w2t[:, m], W2_d[br, :, m])

            wut = {}

            def warmup():
                # run junk matmuls on memset tiles while the first weight
                # DMAs fly, so HAM un-throttles the PE before real work
                wst = actp.tile([128, 128], bf16, tag="wms")
                wmv = actp.tile([128, 256], bf16, tag="wmm")
                nc.gpsimd.memset(wst[:], 0.0)
                nc.gpsimd.memset(wmv[:], 0.0)
                wut["s"] = wst
                wut["m"] = wmv
                for i in range(14):
                    ps = ps1.tile([128, 256], f32, tag="ps", name=f"wu_{i}")
                    nc.tensor.matmul(ps[:], wst[:], wmv[:],
                                     start=True, stop=True)

            def filler(name):
                # keep the PE busy/warm across prologue DMA stalls; fillers
                # use the (still-empty) ps2 pool so they never serialize
                # L1's ps1 slots on drains
                ps = ps2.tile([128, 256], f32, tag="ps", name=name)
                nc.tensor.matmul(ps[:], wut["s"][:], wut["m"][:],
                                 start=True, stop=True)

            def load_px(idx, eng=None, half=False):
                br, bt = iters[idx]
                bsl = slice(bt * BT, (bt + 1) * BT)
                px = pxp.tile([128, BT], bf16, tag="px", name=f"px_{idx}")
                if half:
                    # rows 64-127 are zeros: memset + half-height DMA to
                    # shrink the cold-start critical path
                    nc.vector.memset(px[64:128, :], 0.0)
                    (eng or nc.gpsimd).dma_start(px[0:64, :],
                                                 xE_d[br][0:64, bsl])
                else:
                    (eng or nc.gpsimd).dma_start(px[:], xE_d[br][:, bsl])
                pxs[idx] = px

            H = BT // 2

            def drain(dst, ps, bias, j):
                a, b = (0, H) if j % 2 == 0 else (H, 0)
                nc.scalar.activation(dst[:, a:a + H], ps[:, a:a + H], Relu,
                                     bias=bias, scale=1.0)
                nc.vector.tensor_scalar(dst[:, b:b + H], ps[:, b:b + H],
                                        bias, 0.0, ADD, MAX)

            def drain_relu(dst, ps, j):
                a, b = (0, H) if j % 2 == 0 else (H, 0)
                nc.scalar.activation(dst[:, a:a + H], ps[:, a:a + H], Relu,
                                     bias=0.0, scale=1.0)
                nc.vector.tensor_scalar_max(dst[:, b:b + H], ps[:, b:b + H], 0.0)

            def emit_L1_mm(idx, m):
                br, _ = iters[idx]
                w1t = loaded[br][0]
                if m == 0:
                    h1s[idx] = h1p.tile([128, M1, BT], bf16, tag="h1",
                                        name=f"h1_{idx}")
                ps = ps1.tile([128, BT], f32, tag="ps", name=f"l1_{idx}_{m}")
                nc.tensor.matmul(
                    ps[:], w1t[:, m * 128:(m + 1) * 128], pxs[idx][:],
                    start=True, stop=True,
                )
                drain_relu(h1s[idx][:, m, :], ps[:], m)

            warmup()
            # first inputs ride the otherwise-idle gpsimd queue so they are
            # not stuck behind the branch-0 weight stream
            load_px(0, eng=nc.gpsimd, half=True)
            load_branch(0)   # emits px1/px2 interleaved after W2 m0/m1
            # prologue: iteration 0's L1 runs standalone, with fillers
            # bridging the trickle of first-branch weight DMAs
            for m in range(M1):
                emit_L1_mm(0, m)
                filler(f"f_{m}a")
                filler(f"f_{m}b")

            def emit_head(idx):
                # head of iteration idx, deferred into the following L2
                # window so the L3->head->drain serialization at the
                # iteration boundary disappears into L2 slack
                br, bt = iters[idx]
                wqt, bqt = loaded[br][3], loaded[br][5]
                psq = ps2.tile([NAP, BT], f32, tag="ps", name=f"hd_{idx}")
                for k in range(KH):
                    nc.tensor.matmul(
                        psq[:], wqt[:, k, :], h3s[idx][:, k, :],
                        start=(k == 0), stop=(k == KH - 1),
                    )
                nc.vector.tensor_scalar_add(
                    qss[br][:, bt * BT:(bt + 1) * BT], psq[:], bqt[:]
                )
                if bt == NBT - 1:
                    nc.gpsimd.dma_start(out_d[br], qss[br][:])

            for idx, (br, bt) in enumerate(iters):
                w1t, w2t, w3t, wqt, btile, bqt = loaded[br]
                nxt = idx + 1
                if br + 1 < NB:
                    if bt == 0:
                        load_branch(br + 1)
                    else:
                        load_branch_rest(br + 1)
                if idx + 3 < len(iters):
                    load_px(idx + 3)
                if bt == 0:
                    qss[br] = qsp.tile([NAP, LB], f32, tag="qs", name=f"qs_{br}")
                h1 = h1s[idx]
                h3 = h3p.tile([128, KH, BT], bf16, tag="h3", name=f"h3_{idx}")
                h3s[idx] = h3

                # ---- L2: [2048 -> 1024], m-outer k-inner chains, with next
                # iteration's L1 matmuls injected pairwise mid-chain and the
                # previous iteration's head after chain m1 ----
                for m in range(M2):
                    ps = ps2.tile([128, BT], f32, tag="ps", name=f"l2_{idx}_{m}")
                    for k in range(K2):
                        nc.tensor.matmul(
                            ps[:], w2t[:, m, k, :], h1[:, k, :],
                            start=(k == 0), stop=(k == K2 - 1),
                        )
                        if k == 7 and nxt < len(iters):
                            # paired injection: fewer rhs-switch boundaries
                            emit_L1_mm(nxt, m * 2)
                            emit_L1_mm(nxt, m * 2 + 1)
                    drain(h2[:, m, :], ps[:], btile[:, m:m + 1], m)
                    if m == 1 and idx > 0:
                        emit_head(idx - 1)

                # ---- L3: [1024 -> 512] ----
                for m in range(M3):
                    ps = ps2.tile([128, BT], f32, tag="ps", name=f"l3_{idx}_{m}")
                    for k in range(K3):
                        nc.tensor.matmul(
                            ps[:], w3t[:, m, k, :], h2[:, k, :],
                            start=(k == 0), stop=(k == K3 - 1),
                        )
                    drain(h3[:, m, :], ps[:], btile[:, M2 + m:M2 + m + 1], m)

            emit_head(len(iters) - 1)

    nc.compile()
    _NC_CACHE["nc"] = nc
    return nc


def _pack_weights(W1, b1, W2, b2, W3, b3, Wv, bv, Wa, ba):
    bf = ml_dtypes.bfloat16
    f = np.float32
    # W1 padded to K=128: rows 0-61 = W1, row 62 = b1 (ones row in px),
    # rows 63-127 = 0
    W1p = np.zeros((NB, 128, D1), dtype=bf)
    W1p[:, :D0, :] = W1.astype(bf)
    W1p[:, D0, :] = b1.astype(bf)
    # [br, k*128+p, m*128+c] -> [br, p, m, k, c]
    W2p = np.ascontiguousarray(
        W2.reshape(NB, K2, 128, M2, 128).transpose(0, 2, 3, 1, 4), bf)
    W3p = np.ascontiguousarray(
        W3.reshape(NB, K3, 128, M3, 128).transpose(0, 2, 3, 1, 4), bf)
    # fold dueling head: q = h @ (Wv + Wa - mean(Wa)) + (bv + ba - mean(ba))
    Wq = Wv + Wa - Wa.mean(axis=2, keepdims=True)                # [12, 512, 11]
    bq = bv + ba - ba.mean(axis=1, keepdims=True)                # [12, 11]
    Wq = np.concatenate([Wq, np.zeros((NB, D3, NAP - NA), Wq.dtype)], axis=2)
    bq = np.concatenate([bq, np.zeros((NB, NAP - NA), bq.dtype)], axis=1)
    Wqp = np.ascontiguousarray(
        Wq.reshape(NB, KH, 128, NAP).transpose(0, 2, 1, 3), bf)  # [12,128,4,12]
    bp = np.concatenate(
        [
            b2.reshape(NB, M2, 128).transpose(0, 2, 1),
            b3.reshape(NB, M3, 128).transpose(0, 2, 1),
        ],
        axis=2,
    ).astype(f)                                                  # [12, 128, 12]
    bqp = np.ascontiguousarray(bq.reshape(NB, NAP, 1), f)
    return W1p, W2p, W3p, Wqp, bp, bqp


def kernel(x, W1, b1, W2, b2, W3, b3, Wv, bv, Wa, ba):
    global LAST_RESULT
    from concourse.bass_utils import run_bass_kernel_spmd

    bf = ml_dtypes.bfloat16
    x = np.asarray(x, np.float32)
    args = [np.asarray(a, np.float32) for a in (W1, b1, W2, b2, W3, b3, Wv, bv, Wa, ba)]
    W1p, W2p, W3p, Wqp, bp, bqp = _pack_weights(*args)

    nc = _build_nc()
    in_maps = []
    for c in range(NCORES):
        xl = x[c * LB:(c + 1) * LB]                              # [1024, 249]
        # per-branch padded input: [node(45) | group_br(17) | ones | zeros]
        xE = np.zeros((NB, 128, LB), dtype=bf)
        nodeT = np.ascontiguousarray(xl[:, :NODE].T).astype(bf)  # [45, 1024]
        xE[:, :NODE, :] = nodeT[None]
        for br in range(NB):
            g0 = NODE + GRP * br
            xE[br, NODE:D0, :] = xl[:, g0:g0 + GRP].T.astype(bf)
        xE[:, D0, :] = np.float32(1.0)
        in_maps.append({
            "xE": xE,
            "W1p": W1p, "W2p": W2p, "W3p": W3p, "Wqp": Wqp,
            "bp": bp, "bqp": bqp,
        })

    res = run_bass_kernel_spmd(nc, in_maps, list(range(NCORES)))
    LAST_RESULT = res

    out = np.empty((NB, B, NA), np.float32)
    for c in range(NCORES):
        o = res.results[c]["out"]                                # [12, 12, 1024]
        out[:, c * LB:(c + 1) * LB, :] = o[:, :NA, :].transpose(0, 2, 1)
    return out


# revision 57
# speedup vs baseline: 1.0016x; 1.0016x over previous
"""Trainium2 Bass kernel for nn_BranchingQNetwork (12-branch dueling Q-MLP).

Strategy: data-parallel over batch (8 cores x 1024 rows). bf16 weights and
activations (tolerance 2e-2; bf16 lands ~5e-3). All weights for a branch are
SBUF-resident and double-buffered, streamed from HBM once per core, so the PE
never waits on weight DMAs. L2/L3 run m-outer/k-inner accumulation chains so
PSUM drains overlap the next chain's matmuls. The next iteration's L1
matmuls are interleaved into the current L2 window (h1 double-buffered). The
per-branch input px is host-packed as [node | group_br | ones | zeros] with
K padded to 128 so every matmul has a uniform [128, 128] stationary operand
(keeps the PE weight-buffer pull-ahead clean) and the L1 bias rides in the
ones row. The dueling head is folded host-side into a single [512, 12]
matrix and computed action-major (stationary = Wq k-tiles, moving = h3); the
[12, LB] output is transposed back to batch-major on the host.
"""
import sys

sys.path.insert(0, "/opt/trn_rl_repo")

import numpy as np
import ml_dtypes

# problem dims (hardcoded per harness contract)
B = 8192
OBS = 249
NB = 12
NA = 11
NODE = 45
GRP = 17
D0 = 62
D1 = 2048
D2 = 1024
D3 = 512

NCORES = 8
LB = B // NCORES     # local batch per core
BT = 512             # batch tile (one PSUM bank of fp32)
NBT = LB // BT
M1 = D1 // 128       # 16 L1 output tiles
M2 = D2 // 128       # 8
K2 = D1 // 128       # 16
M3 = D3 // 128       # 4
K3 = D2 // 128       # 8
KH = D3 // 128       # 4
NAP = 12             # head width padded even

_NC_CACHE = {}
LAST_RESULT = None


def _build_nc():
    if "nc" in _NC_CACHE:
        return _NC_CACHE["nc"]
    from concourse import bacc
    import concourse.mybir as mybir
    import concourse.tile as tile

    f32 = mybir.dt.float32
    bf16 = mybir.dt.bfloat16
    Relu = mybir.ActivationFunctionType.Relu
    ADD = mybir.AluOpType.add
    MAX = mybir.AluOpType.max

    nc = bacc.Bacc("TRN2")

    xE_d = nc.declare_dram_parameter("xE", [NB, 128, LB], bf16, isOutput=False)
    W1_d = nc.declare_dram_parameter("W1p", [NB, 128, D1], bf16, isOutput=False)
    W2_d = nc.declare_dram_parameter("W2p", [NB, 128, M2, K2, 128], bf16, isOutput=False)
    W3_d = nc.declare_dram_parameter("W3p", [NB, 128, M3, K3, 128], bf16, isOutput=False)
    Wq_d = nc.declare_dram_parameter("Wqp", [NB, 128, KH, NAP], bf16, isOutput=False)
    b_d = nc.declare_dram_parameter("bp", [NB, 128, M2 + M3], f32, isOutput=False)
    bq_d = nc.declare_dram_parameter("bqp", [NB, NAP, 1], f32, isOutput=False)
    out_d = nc.declare_dram_parameter("out", [NB, NAP, LB], f32, isOutput=True)

    with tile.TileContext(nc) as tc:
        with (
            tc.tile_pool(name="w1p", bufs=2) as w1p,
            tc.tile_pool(name="w2p", bufs=2) as w2p,
            tc.tile_pool(name="w3p", bufs=2) as w3p,
            tc.tile_pool(name="wqp", bufs=2) as wqp,
            tc.tile_pool(name="bbp", bufs=2) as bbp,
            tc.tile_pool(name="pxp", bufs=4) as pxp,
            tc.tile_pool(name="h1p", bufs=2) as h1p,
            tc.tile_pool(name="h3p", bufs=2) as h3p,
            tc.tile_pool(name="actp", bufs=1) as actp,
            tc.tile_pool(name="qsp", bufs=2) as qsp,
            tc.tile_pool(name="ps1", bufs=3, space="PSUM") as ps1,
            tc.tile_pool(name="ps2", bufs=5, space="PSUM") as ps2,
        ):
            h2 = actp.tile([128, K3, BT], bf16, tag="h2")

            loaded = {}
            pxs = {}
            h1s = {}
            h3s = {}
            qss = {}
            iters = [(br, bt) for br in range(NB) for bt in range(NBT)]

            def load_branch(br):
                w1t = w1p.tile([128, D1], bf16, tag="w1", name=f"w1_{br}")
                if br == 0:
                    # rows 64-127 of the padded W1 are zeros: memset them
                    # on-chip and DMA only rows 0-63, halving the startup
                    # critical-path bytes. Column-chunked; chunk 0 rides the
                    # (fast cold-start) gpsimd SWDGE path so the first L1
                    # matmul gates on 64KB of DMA.
                    nc.vector.memset(w1t[64:128, :], 0.0)
                    for c in range(4):
                        eng = nc.gpsimd if c == 0 else nc.sync
                        eng.dma_start(
                            w1t[0:64, c * 512:(c + 1) * 512],
                            W1_d[br][0:64, c * 512:(c + 1) * 512],
                        )
                else:
                    nc.scalar.dma_start(w1t[:], W1_d[br])
                btile = bbp.tile([128, M2 + M3], f32, tag="b", name=f"b_{br}")
                nc.scalar.dma_start(btile[:], b_d[br])
                bqt = bbp.tile([NAP, 1], f32, tag="bq", name=f"bq_{br}")
                nc.scalar.dma_start(bqt[:], bq_d[br])
                wqt = wqp.tile([128, KH, NAP], bf16, tag="wq", name=f"wq_{br}")
                nc.scalar.dma_start(wqt[:], Wq_d[br])
                w2t = w2p.tile([128, M2, K2, 128], bf16, tag="w2", name=f"w2_{br}")
                nm = M2 if br == 0 else M2 // 2
                for m in range(nm):
                    eng = nc.gpsimd if (br == 0 and m < 2) else nc.sync
                    eng.dma_start(w2t[:, m], W2_d[br, :, m])
                    if br == 0 and m < 2:
                        # interleave the early px loads behind each critical
                        # W2 chunk on the cold-start gpsimd ring
                        load_px(m + 1, eng=nc.gpsimd, half=True)
                w3t = w3p.tile([128, M3, K3, 128], bf16, tag="w3", name=f"w3_{br}")
                # branch 0's W3 rides the sync queue behind W2 so it doesn't
                # steal HBM bandwidth from the critical first L1/L2 weights
                (nc.sync if br == 0 else nc.gpsimd).dma_start(w3t[:], W3_d[br])
                loaded[br] = (w1t, w2t, w3t, wqt, btile, bqt)

            def load_branch_rest(br):
                # second half of the W2 prefetch, deferred one iteration to
                # smooth the HBM burst (the stack is shared with a neighbor
                # core running the same phase-drifting schedule)
                w2t = loaded[br][1]
                for m in range(M2 // 2, M2):
                    nc.sync.dma_start(w2t[:, m], W2_d[br, :, m])

            wut = {}

            def warmup():
                # run junk matmuls on memset tiles while the first weight
                # DMAs fly, so HAM un-throttles the PE before real work
                wst = actp.tile([128, 128], bf16, tag="wms")
                wmv = actp.tile([128, 256], bf16, tag="wmm")
                nc.gpsimd.memset(wst[:], 0.0)
                nc.gpsimd.memset(wmv[:], 0.0)
                wut["s"] = wst
                wut["m"] = wmv
                for i in range(14):
                    ps = ps1.tile([128, 256], f32, tag="ps", name=f"wu_{i}")
                    nc.tensor.matmul(ps[:], wst[:], wmv[:],
                                     start=True, stop=True)

            def filler(name):
                # keep the PE busy/warm across prologue DMA stalls; fillers
                # use the (still-empty) ps2 pool so they never serialize
                # L1's ps1 slots on drains
                ps = ps2.tile([128, 256], f32, tag="ps", name=name)
                nc.tensor.matmul(ps[:], wut["s"][:], wut["m"][:],
                                 start=True, stop=True)

            def load_px(idx, eng=None, half=False):
                br, bt = iters[idx]
                bsl = slice(bt * BT, (bt + 1) * BT)
                px = pxp.tile([128, BT], bf16, tag="px", name=f"px_{idx}")
                if half:
                    # rows 64-127 are zeros: memset + half-height DMA to
                    # shrink the cold-start critical path
                    nc.vector.memset(px[64:128, :], 0.0)
                    (eng or nc.gpsimd).dma_start(px[0:64, :],
                                                 xE_d[br][0:64, bsl])
                else:
                    (eng or nc.gpsimd).dma_start(px[:], xE_d[br][:, bsl])
                pxs[idx] = px

            H = BT // 2

            def drain(dst, ps, bias, j):
                a, b = (0, H) if j % 2 == 0 else (H, 0)
                nc.scalar.activation(dst[:, a:a + H], ps[:, a:a + H], Relu,
                                     bias=bias, scale=1.0)
                nc.vector.tensor_scalar(dst[:, b:b + H], ps[:, b:b + H],
                                        bias, 0.0, ADD, MAX)

            def drain_relu(dst, ps, j):
                a, b = (0, H) if j % 2 == 0 else (H, 0)
                nc.scalar.activation(dst[:, a:a + H], ps[:, a:a + H], Relu,
                                     bias=0.0, scale=1.0)
                nc.vector.tensor_scalar_max(dst[:, b:b + H], ps[:, b:b + H], 0.0)

            def emit_L1_mm(idx, m):
                br, _ = iters[idx]
                w1t = loaded[br][0]
                if m == 0:
                    h1s[idx] = h1p.tile([128, M1, BT], bf16, tag="h1",
                                        name=f"h1_{idx}")
                ps = ps1.tile([128, BT], f32, tag="ps", name=f"l1_{idx}_{m}")
                nc.tensor.matmul(
                    ps[:], w1t[:, m * 128:(m + 1) * 128], pxs[idx][:],
                    start=True, stop=True,
                )
                drain_relu(h1s[idx][:, m, :], ps[:], m)

            warmup()
            # first inputs ride the otherwise-idle gpsimd queue so they are
            # not stuck behind the branch-0 weight stream
            load_px(0, eng=nc.gpsimd, half=True)
            load_branch(0)   # emits px1/px2 interleaved after W2 m0/m1
            # prologue: iteration 0's L1 runs standalone, with fillers
            # bridging the trickle of first-branch weight DMAs
            for m in range(M1):
                emit_L1_mm(0, m)
                filler(f"f_{m}a")
                filler(f"f_{m}b")

            def emit_head(idx):
                # head of iteration idx, deferred into the following L2
                # window so the L3->head->drain serialization at the
                # iteration boundary disappears into L2 slack
                br, bt = iters[idx]
                wqt, bqt = loaded[br][3], loaded[br][5]
                psq = ps2.tile([NAP, BT], f32, tag="ps", name=f"hd_{idx}")
                for k in range(KH):
                    nc.tensor.matmul(
                        psq[:], wqt[:, k, :], h3s[idx][:, k, :],
                        start=(k == 0), stop=(k == KH - 1),
                    )
                nc.vector.tensor_scalar_add(
                    qss[br][:, bt * BT:(bt + 1) * BT], psq[:], bqt[:]
                )
                # per-bt half DMA: the final transfer before teardown is
                # half as large, and each half ships as soon as it's ready
                nc.gpsimd.dma_start(out_d[br][:, bt * BT:(bt + 1) * BT],
                                    qss[br][:, bt * BT:(bt + 1) * BT])

            for idx, (br, bt) in enumerate(iters):
                w1t, w2t, w3t, wqt, btile, bqt = loaded[br]
                nxt = idx + 1
                if br + 1 < NB:
                    if bt == 0:
                        load_branch(br + 1)
                    else:
                        load_branch_rest(br + 1)
                if idx + 3 < len(iters):
                    load_px(idx + 3)
                if bt == 0:
                    qss[br] = qsp.tile([NAP, LB], f32, tag="qs", name=f"qs_{br}")
                h1 = h1s[idx]
                h3 = h3p.tile([128, KH, BT], bf16, tag="h3", name=f"h3_{idx}")
                h3s[idx] = h3

                # ---- L2: [2048 -> 1024], m-outer k-inner chains, with next
                # iteration's L1 matmuls injected pairwise mid-chain and the
                # previous iteration's head after chain m1 ----
                for m in range(M2):
                    ps = ps2.tile([128, BT], f32, tag="ps", name=f"l2_{idx}_{m}")
                    for k in range(K2):
                        nc.tensor.matmul(
                            ps[:], w2t[:, m, k, :], h1[:, k, :],
                            start=(k == 0), stop=(k == K2 - 1),
                        )
                        if k == 7 and nxt < len(iters):
                            # paired injection: fewer rhs-switch boundaries
                            emit_L1_mm(nxt, m * 2)
                            emit_L1_mm(nxt, m * 2 + 1)
                    drain(h2[:, m, :], ps[:], btile[:, m:m + 1], m)
                    if m == 1 and idx > 0:
                        emit_head(idx - 1)

                # ---- L3: [1024 -> 512] ----
                for m in range(M3):
                    ps = ps2.tile([128, BT], f32, tag="ps", name=f"l3_{idx}_{m}")
                    for k in range(K3):
                        nc.tensor.matmul(
                            ps[:], w3t[:, m, k, :], h2[:, k, :],
                            start=(k == 0), stop=(k == K3 - 1),
                        )
                    drain(h3[:, m, :], ps[:], btile[:, M2 + m:M2 + m + 1], m)

            emit_head(len(iters) - 1)

    nc.compile()
    _NC_CACHE["nc"] = nc
    return nc


def _pack_weights(W1, b1, W2, b2, W3, b3, Wv, bv, Wa, ba):
    bf = ml_dtypes.bfloat16
    f = np.float32
    # W1 padded to K=128: rows 0-61 = W1, row 62 = b1 (ones row in px),
    # rows 63-127 = 0
    W1p = np.zeros((NB, 128, D1), dtype=bf)
    W1p[:, :D0, :] = W1.astype(bf)
    W1p[:, D0, :] = b1.astype(bf)
    # [br, k*128+p, m*128+c] -> [br, p, m, k, c]
    W2p = np.ascontiguousarray(
        W2.reshape(NB, K2, 128, M2, 128).transpose(0, 2, 3, 1, 4), bf)
    W3p = np.ascontiguousarray(
        W3.reshape(NB, K3, 128, M3, 128).transpose(0, 2, 3, 1, 4), bf)
    # fold dueling head: q = h @ (Wv + Wa - mean(Wa)) + (bv + ba - mean(ba))
    Wq = Wv + Wa - Wa.mean(axis=2, keepdims=True)                # [12, 512, 11]
    bq = bv + ba - ba.mean(axis=1, keepdims=True)                # [12, 11]
    Wq = np.concatenate([Wq, np.zeros((NB, D3, NAP - NA), Wq.dtype)], axis=2)
    bq = np.concatenate([bq, np.zeros((NB, NAP - NA), bq.dtype)], axis=1)
    Wqp = np.ascontiguousarray(
        Wq.reshape(NB, KH, 128, NAP).transpose(0, 2, 1, 3), bf)  # [12,128,4,12]
    bp = np.concatenate(
        [
            b2.reshape(NB, M2, 128).transpose(0, 2, 1),
            b3.reshape(NB, M3, 128).transpose(0, 2, 1),
        ],
        axis=2,
    ).astype(f)                                                  # [12, 128, 12]
    bqp = np.ascontiguousarray(bq.reshape(NB, NAP, 1), f)
    return W1p, W2p, W3p, Wqp, bp, bqp


def kernel(x, W1, b1, W2, b2, W3, b3, Wv, bv, Wa, ba):
    global LAST_RESULT
    from concourse.bass_utils import run_bass_kernel_spmd

    bf = ml_dtypes.bfloat16
    x = np.asarray(x, np.float32)
    args = [np.asarray(a, np.float32) for a in (W1, b1, W2, b2, W3, b3, Wv, bv, Wa, ba)]
    W1p, W2p, W3p, Wqp, bp, bqp = _pack_weights(*args)

    nc = _build_nc()
    in_maps = []
    for c in range(NCORES):
        xl = x[c * LB:(c + 1) * LB]                              # [1024, 249]
        # per-branch padded input: [node(45) | group_br(17) | ones | zeros]
        xE = np.zeros((NB, 128, LB), dtype=bf)
        nodeT = np.ascontiguousarray(xl[:, :NODE].T).astype(bf)  # [45, 1024]
        xE[:, :NODE, :] = nodeT[None]
        for br in range(NB):
            g0 = NODE + GRP * br
            xE[br, NODE:D0, :] = xl[:, g0:g0 + GRP].T.astype(bf)
        xE[:, D0, :] = np.float32(1.0)
        in_maps.append({
            "xE": xE,
            "W1p": W1p, "W2p": W2p, "W3p": W3p, "Wqp": Wqp,
            "bp": bp, "bqp": bqp,
        })

    res = run_bass_kernel_spmd(nc, in_maps, list(range(NCORES)))
    LAST_RESULT = res

    out = np.empty((NB, B, NA), np.float32)
    for c in range(NCORES):
        o = res.results[c]["out"]                                # [12, 12, 1024]
        out[:, c * LB:(c + 1) * LB, :] = o[:, :NA, :].transpose(0, 2, 1)
    return out


# revision 60
# speedup vs baseline: 1.0065x; 1.0049x over previous
"""Trainium2 Bass kernel for nn_BranchingQNetwork (12-branch dueling Q-MLP).

Strategy: data-parallel over batch (8 cores x 1024 rows). bf16 weights and
activations (tolerance 2e-2; bf16 lands ~5e-3). All weights for a branch are
SBUF-resident and double-buffered, streamed from HBM once per core, so the PE
never waits on weight DMAs. L2/L3 run m-outer/k-inner accumulation chains so
PSUM drains overlap the next chain's matmuls. The next iteration's L1
matmuls are interleaved into the current L2 window (h1 double-buffered). The
per-branch input px is host-packed as [node | group_br | ones | zeros] with
K padded to 128 so every matmul has a uniform [128, 128] stationary operand
(keeps the PE weight-buffer pull-ahead clean) and the L1 bias rides in the
ones row. The dueling head is folded host-side into a single [512, 12]
matrix and computed action-major (stationary = Wq k-tiles, moving = h3); the
[12, LB] output is transposed back to batch-major on the host.
"""
import sys

sys.path.insert(0, "/opt/trn_rl_repo")

import numpy as np
import ml_dtypes

# problem dims (hardcoded per harness contract)
B = 8192
OBS = 249
NB = 12
NA = 11
NODE = 45
GRP = 17
D0 = 62
D1 = 2048
D2 = 1024
D3 = 512

NCORES = 8
LB = B // NCORES     # local batch per core
BT = 512             # batch tile (one PSUM bank of fp32)
NBT = LB // BT
M1 = D1 // 128       # 16 L1 output tiles
M2 = D2 // 128       # 8
K2 = D1 // 128       # 16
M3 = D3 // 128       # 4
K3 = D2 // 128       # 8
KH = D3 // 128       # 4
NAP = 12             # head width padded even

_NC_CACHE = {}
LAST_RESULT = None


def _build_nc():
    if "nc" in _NC_CACHE:
        return _NC_CACHE["nc"]
    from concourse import bacc
    import concourse.mybir as mybir
    import concourse.tile as tile

    f32 = mybir.dt.float32
    bf16 = mybir.dt.bfloat16
    Relu = mybir.ActivationFunctionType.Relu
    ADD = mybir.AluOpType.add
    MAX = mybir.AluOpType.max

    nc = bacc.Bacc("TRN2")

    xE_d = nc.declare_dram_parameter("xE", [NB, 128, LB], bf16, isOutput=False)
    W1_d = nc.declare_dram_parameter("W1p", [NB, 128, D1], bf16, isOutput=False)
    W2_d = nc.declare_dram_parameter("W2p", [NB, 128, M2, K2, 128], bf16, isOutput=False)
    W3_d = nc.declare_dram_parameter("W3p", [NB, 128, M3, K3, 128], bf16, isOutput=False)
    Wq_d = nc.declare_dram_parameter("Wqp", [NB, 128, KH, NAP], bf16, isOutput=False)
    b_d = nc.declare_dram_parameter("bp", [NB, 128, M2 + M3], f32, isOutput=False)
    bq_d = nc.declare_dram_parameter("bqp", [NB, NAP, 1], f32, isOutput=False)
    out_d = nc.declare_dram_parameter("out", [NB, NAP, LB], f32, isOutput=True)

    with tile.TileContext(nc) as tc:
        with (
            tc.tile_pool(name="w1p", bufs=2) as w1p,
            tc.tile_pool(name="w2p", bufs=2) as w2p,
            tc.tile_pool(name="w3p", bufs=2) as w3p,
            tc.tile_pool(name="wqp", bufs=2) as wqp,
            tc.tile_pool(name="bbp", bufs=2) as bbp,
            tc.tile_pool(name="pxp", bufs=4) as pxp,
            tc.tile_pool(name="h1p", bufs=2) as h1p,
            tc.tile_pool(name="h3p", bufs=2) as h3p,
            tc.tile_pool(name="actp", bufs=1) as actp,
            tc.tile_pool(name="qsp", bufs=2) as qsp,
            tc.tile_pool(name="ps1", bufs=3, space="PSUM") as ps1,
            tc.tile_pool(name="ps2", bufs=5, space="PSUM") as ps2,
        ):
            h2 = actp.tile([128, K3, BT], bf16, tag="h2")

            loaded = {}
            pxs = {}
            h1s = {}
            h3s = {}
            qss = {}
            iters = [(br, bt) for br in range(NB) for bt in range(NBT)]

            def load_branch(br):
                w1t = w1p.tile([128, D1], bf16, tag="w1", name=f"w1_{br}")
                if br == 0:
                    # rows 64-127 of the padded W1 are zeros: memset them
                    # on-chip and DMA only rows 0-63, halving the startup
                    # critical-path bytes. Column-chunked; chunk 0 rides the
                    # (fast cold-start) gpsimd SWDGE path so the first L1
                    # matmul gates on 64KB of DMA.
                    nc.vector.memset(w1t[64:128, :], 0.0)
                    for c in range(4):
                        eng = nc.gpsimd if c == 0 else nc.sync
                        eng.dma_start(
                            w1t[0:64, c * 512:(c + 1) * 512],
                            W1_d[br][0:64, c * 512:(c + 1) * 512],
                        )
                else:
                    nc.scalar.dma_start(w1t[:], W1_d[br])
                btile = bbp.tile([128, M2 + M3], f32, tag="b", name=f"b_{br}")
                nc.scalar.dma_start(btile[:], b_d[br])
                bqt = bbp.tile([NAP, 1], f32, tag="bq", name=f"bq_{br}")
                nc.scalar.dma_start(bqt[:], bq_d[br])
                wqt = wqp.tile([128, KH, NAP], bf16, tag="wq", name=f"wq_{br}")
                nc.scalar.dma_start(wqt[:], Wq_d[br])
                w2t = w2p.tile([128, M2, K2, 128], bf16, tag="w2", name=f"w2_{br}")
                if br == 0:
                    for m in range(M2):
                        eng = nc.gpsimd if m < 2 else nc.sync
                        eng.dma_start(w2t[:, m], W2_d[br, :, m])
                        if m < 2:
                            # interleave the early px loads behind each
                            # critical W2 chunk on the cold-start ring
                            load_px(m + 1, eng=nc.gpsimd, half=True)
                else:
                    # one 2MB transfer: fewer DMA-completion semaphore
                    # broadcasts interrupting the PE issue stream
                    nc.sync.dma_start(w2t[:, 0:M2 // 2], W2_d[br, :, 0:M2 // 2])
                w3t = w3p.tile([128, M3, K3, 128], bf16, tag="w3", name=f"w3_{br}")
                # branch 0's W3 rides the sync queue behind W2 so it doesn't
                # steal HBM bandwidth from the critical first L1/L2 weights
                (nc.sync if br == 0 else nc.gpsimd).dma_start(w3t[:], W3_d[br])
                loaded[br] = (w1t, w2t, w3t, wqt, btile, bqt)

            def load_branch_rest(br):
                # second half of the W2 prefetch, deferred one iteration to
                # smooth the HBM burst (the stack is shared with a neighbor
                # core running the same phase-drifting schedule)
                w2t = loaded[br][1]
                nc.sync.dma_start(w2t[:, M2 // 2:M2], W2_d[br, :, M2 // 2:M2])

            wut = {}

            def warmup():
                # run junk matmuls on memset tiles while the first weight
                # DMAs fly, so HAM un-throttles the PE before real work
                wst = actp.tile([128, 128], bf16, tag="wms")
                wmv = actp.tile([128, 256], bf16, tag="wmm")
                nc.gpsimd.memset(wst[:], 0.0)
                nc.gpsimd.memset(wmv[:], 0.0)
                wut["s"] = wst
                wut["m"] = wmv
                for i in range(14):
                    ps = ps1.tile([128, 256], f32, tag="ps", name=f"wu_{i}")
                    nc.tensor.matmul(ps[:], wst[:], wmv[:],
                                     start=True, stop=True)

            def filler(name):
                # keep the PE busy/warm across prologue DMA stalls; fillers
                # use the (still-empty) ps2 pool so they never serialize
                # L1's ps1 slots on drains
                ps = ps2.tile([128, 256], f32, tag="ps", name=name)
                nc.tensor.matmul(ps[:], wut["s"][:], wut["m"][:],
                                 start=True, stop=True)

            def load_px(idx, eng=None, half=False):
                br, bt = iters[idx]
                bsl = slice(bt * BT, (bt + 1) * BT)
                px = pxp.tile([128, BT], bf16, tag="px", name=f"px_{idx}")
                if half:
                    # rows 64-127 are zeros: memset + half-height DMA to
                    # shrink the cold-start critical path
                    nc.vector.memset(px[64:128, :], 0.0)
                    (eng or nc.gpsimd).dma_start(px[0:64, :],
                                                 xE_d[br][0:64, bsl])
                else:
                    (eng or nc.gpsimd).dma_start(px[:], xE_d[br][:, bsl])
                pxs[idx] = px

            H = BT // 2

            def drain(dst, ps, bias, j):
                a, b = (0, H) if j % 2 == 0 else (H, 0)
                nc.scalar.activation(dst[:, a:a + H], ps[:, a:a + H], Relu,
                                     bias=bias, scale=1.0)
                nc.vector.tensor_scalar(dst[:, b:b + H], ps[:, b:b + H],
                                        bias, 0.0, ADD, MAX)

            def drain_relu(dst, ps, j):
                a, b = (0, H) if j % 2 == 0 else (H, 0)
                nc.scalar.activation(dst[:, a:a + H], ps[:, a:a + H], Relu,
                                     bias=0.0, scale=1.0)
                nc.vector.tensor_scalar_max(dst[:, b:b + H], ps[:, b:b + H], 0.0)

            def emit_L1_mm(idx, m):
                br, _ = iters[idx]
                w1t = loaded[br][0]
                if m == 0:
                    h1s[idx] = h1p.tile([128, M1, BT], bf16, tag="h1",
                                        name=f"h1_{idx}")
                ps = ps1.tile([128, BT], f32, tag="ps", name=f"l1_{idx}_{m}")
                nc.tensor.matmul(
                    ps[:], w1t[:, m * 128:(m + 1) * 128], pxs[idx][:],
                    start=True, stop=True,
                )
                drain_relu(h1s[idx][:, m, :], ps[:], m)

            warmup()
            # first inputs ride the otherwise-idle gpsimd queue so they are
            # not stuck behind the branch-0 weight stream
            load_px(0, eng=nc.gpsimd, half=True)
            load_branch(0)   # emits px1/px2 interleaved after W2 m0/m1
            # prologue: iteration 0's L1 runs standalone, with fillers
            # bridging the trickle of first-branch weight DMAs
            for m in range(M1):
                emit_L1_mm(0, m)
                filler(f"f_{m}a")
                filler(f"f_{m}b")

            def emit_head(idx):
                # head of iteration idx, deferred into the following L2
                # window so the L3->head->drain serialization at the
                # iteration boundary disappears into L2 slack
                br, bt = iters[idx]
                wqt, bqt = loaded[br][3], loaded[br][5]
                psq = ps2.tile([NAP, BT], f32, tag="ps", name=f"hd_{idx}")
                for k in range(KH):
                    nc.tensor.matmul(
                        psq[:], wqt[:, k, :], h3s[idx][:, k, :],
                        start=(k == 0), stop=(k == KH - 1),
                    )
                nc.vector.tensor_scalar_add(
                    qss[br][:, bt * BT:(bt + 1) * BT], psq[:], bqt[:]
                )
                # per-bt half DMA: the final transfer before teardown is
                # half as large, and each half ships as soon as it's ready;
                # the very last one rides sync so gpsimd's teardown drain
                # is not serialized behind it
                eng = nc.sync if idx == len(iters) - 1 else nc.gpsimd
                eng.dma_start(out_d[br][:, bt * BT:(bt + 1) * BT],
                              qss[br][:, bt * BT:(bt + 1) * BT])

            for idx, (br, bt) in enumerate(iters):
                w1t, w2t, w3t, wqt, btile, bqt = loaded[br]
                nxt = idx + 1
                if br + 1 < NB:
                    if bt == 0:
                        load_branch(br + 1)
                    else:
                        load_branch_rest(br + 1)
                if idx + 3 < len(iters):
                    load_px(idx + 3)
                if bt == 0:
                    qss[br] = qsp.tile([NAP, LB], f32, tag="qs", name=f"qs_{br}")
                h1 = h1s[idx]
                h3 = h3p.tile([128, KH, BT], bf16, tag="h3", name=f"h3_{idx}")
                h3s[idx] = h3

                # ---- L2: [2048 -> 1024], m-outer k-inner chains, with next
                # iteration's L1 matmuls injected pairwise mid-chain and the
                # previous iteration's head after chain m1 ----
                for m in range(M2):
                    ps = ps2.tile([128, BT], f32, tag="ps", name=f"l2_{idx}_{m}")
                    for k in range(K2):
                        nc.tensor.matmul(
                            ps[:], w2t[:, m, k, :], h1[:, k, :],
                            start=(k == 0), stop=(k == K2 - 1),
                        )
                        if k == 7 and nxt < len(iters):
                            # paired injection: fewer rhs-switch boundaries
                            emit_L1_mm(nxt, m * 2)
                            emit_L1_mm(nxt, m * 2 + 1)
                    drain(h2[:, m, :], ps[:], btile[:, m:m + 1], m)
                    if m == 1 and idx > 0:
                        emit_head(idx - 1)

                # ---- L3: [1024 -> 512] ----
                for m in range(M3):
                    ps = ps2.tile([128, BT], f32, tag="ps", name=f"l3_{idx}_{m}")
                    for k in range(K3):
                        nc.tensor.matmul(
                            ps[:], w3t[:, m, k, :], h2[:, k, :],
                            start=(k == 0), stop=(k == K3 - 1),
                        )
                    drain(h3[:, m, :], ps[:], btile[:, M2 + m:M2 + m + 1], m)

            emit_head(len(iters) - 1)

    nc.compile()
    _NC_CACHE["nc"] = nc
    return nc


def _pack_weights(W1, b1, W2, b2, W3, b3, Wv, bv, Wa, ba):
    bf = ml_dtypes.bfloat16
    f = np.float32
    # W1 padded to K=128: rows 0-61 = W1, row 62 = b1 (ones row in px),
    # rows 63-127 = 0
    W1p = np.zeros((NB, 128, D1), dtype=bf)
    W1p[:, :D0, :] = W1.astype(bf)
    W1p[:, D0, :] = b1.astype(bf)
    # [br, k*128+p, m*128+c] -> [br, p, m, k, c]
    W2p = np.ascontiguousarray(
        W2.reshape(NB, K2, 128, M2, 128).transpose(0, 2, 3, 1, 4), bf)
    W3p = np.ascontiguousarray(
        W3.reshape(NB, K3, 128, M3, 128).transpose(0, 2, 3, 1, 4), bf)
    # fold dueling head: q = h @ (Wv + Wa - mean(Wa)) + (bv + ba - mean(ba))
    Wq = Wv + Wa - Wa.mean(axis=2, keepdims=True)                # [12, 512, 11]
    bq = bv + ba - ba.mean(axis=1, keepdims=True)                # [12, 11]
    Wq = np.concatenate([Wq, np.zeros((NB, D3, NAP - NA), Wq.dtype)], axis=2)
    bq = np.concatenate([bq, np.zeros((NB, NAP - NA), bq.dtype)], axis=1)
    Wqp = np.ascontiguousarray(
        Wq.reshape(NB, KH, 128, NAP).transpose(0, 2, 1, 3), bf)  # [12,128,4,12]
    bp = np.concatenate(
        [
            b2.reshape(NB, M2, 128).transpose(0, 2, 1),
            b3.reshape(NB, M3, 128).transpose(0, 2, 1),
        ],
        axis=2,
    ).astype(f)                                                  # [12, 128, 12]
    bqp = np.ascontiguousarray(bq.reshape(NB, NAP, 1), f)
    return W1p, W2p, W3p, Wqp, bp, bqp


def kernel(x, W1, b1, W2, b2, W3, b3, Wv, bv, Wa, ba):
    global LAST_RESULT
    from concourse.bass_utils import run_bass_kernel_spmd

    bf = ml_dtypes.bfloat16
    x = np.asarray(x, np.float32)
    args = [np.asarray(a, np.float32) for a in (W1, b1, W2, b2, W3, b3, Wv, bv, Wa, ba)]
    W1p, W2p, W3p, Wqp, bp, bqp = _pack_weights(*args)

    nc = _build_nc()
    in_maps = []
    for c in range(NCORES):
        xl = x[c * LB:(c + 1) * LB]                              # [1024, 249]
        # per-branch padded input: [node(45) | group_br(17) | ones | zeros]
        xE = np.zeros((NB, 128, LB), dtype=bf)
        nodeT = np.ascontiguousarray(xl[:, :NODE].T).astype(bf)  # [45, 1024]
        xE[:, :NODE, :] = nodeT[None]
        for br in range(NB):
            g0 = NODE + GRP * br
            xE[br, NODE:D0, :] = xl[:, g0:g0 + GRP].T.astype(bf)
        xE[:, D0, :] = np.float32(1.0)
        in_maps.append({
            "xE": xE,
            "W1p": W1p, "W2p": W2p, "W3p": W3p, "Wqp": Wqp,
            "bp": bp, "bqp": bqp,
        })

    res = run_bass_kernel_spmd(nc, in_maps, list(range(NCORES)))
    LAST_RESULT = res

    out = np.empty((NB, B, NA), np.float32)
    for c in range(NCORES):
        o = res.results[c]["out"]                                # [12, 12, 1024]
        out[:, c * LB:(c + 1) * LB, :] = o[:, :NA, :].transpose(0, 2, 1)
    return out
